# revision 15
# baseline (speedup 1.0000x reference)
"""Tensor-parallel causal attention block for Trainium2 (8 NeuronCores).

Sharding: tensor-parallel across heads for QKV+attention (2 heads/core),
then an AllToAll (fp16 payload, 4MB/core) to switch to row-parallel for
the output projection — much cheaper than the AllReduce the module's
TPLinear layout implies (64MB/core).

Dataflow per core: x^T is pre-transposed on the host so the C-contraction
sits on the partition axis. Q,K are produced transposed [d,t] with RoPE
fused into the PSUM eviction (cos/sin tables host-precomputed); V in
natural [t,d] layout. Scores are computed transposed (S^T = K·Q) so the
PV matmul needs no on-chip transposes at all. Softmax skips the
max-subtraction (scores are O(5) here, exp is fp32-safe), masks causality
with a host-built 0/1 tile, and computes scores/exp/PV only on the
causally-live column range of each diagonal chunk.

The softmax denominator is accumulated on the vector engine (fp16 adds
into an SBUF tile; sums stay < 1e4, well inside fp16 range) and reduced
across partitions with a single ones-vector matmul per query group —
keeping the PE out of the per-chunk reduction entirely. Normalization is
folded into the eviction via reciprocal + a rank-1 broadcast matmul.

The on-chip AllToAll's cost is almost entirely rendezvous latency, not
bandwidth, so multi-rep programs software-pipeline it away: rep k's
collectives are issued at the end of its attention and rep k's output
projection is emitted after rep k+1's first batch, ~130us of compute
later. Single-rep programs (the kernel() path) run the phases serially.

All matmul inputs are fp16 (inputs cast on host, intermediates written as
fp16 by the evicting engine); accumulation stays fp32 in PSUM. Score
tiles are paired two-per-PSUM-allocation so one exp covers ~1024 columns.
"""
import numpy as np

import concourse.bass as bass
import concourse.tile as tile
import concourse.mybir as mybir
from concourse.bass_utils import run_bass_kernel_spmd

N_CORES = 8
B, T, C = 4, 2048, 2048
H = 16                 # total heads
HPC = H // N_CORES     # heads per core = 2
D = C // H             # head dim = 128
P = 128                # partitions
TG = 512               # t-group (moving free dim)
NTG = T // TG          # 4 groups per batch
NCC = C // P           # 16 contraction chunks
NSLICE = B * T // N_CORES  # 1024 output rows per core
NSPLIT = 1             # A2A token segments per head
SEG = NSLICE // NSPLIT
TGP = 512              # projection t-group

FP = mybir.dt.float32
FPR = mybir.dt.float32r
FP16 = mybir.dt.float16
EXP = mybir.ActivationFunctionType.Exp
SCALE = 1.0 / float(np.sqrt(D))

# ---------------------------------------------------------------------------
# Workaround: this container's walrus rejects >1 sync-wait per instruction.
# Hoist extras onto preceding same-engine NoOps (engine streams are in-order).
# ---------------------------------------------------------------------------
from concourse.vector_clock import ScopedClock


def _fixup_multiwaits(nc):
    moved = 0
    for fn in nc.m.functions:
        for bb in fn.blocks:
            insts = bb.instructions
            if not any(
                i.sync_info and i.sync_info.on_wait and len(i.sync_info.on_wait) > 1
                for i in insts
            ):
                continue
            new_insts = []
            for ins in insts:
                si = ins.sync_info
                if si is not None and si.on_wait and len(si.on_wait) > 1:
                    extra, keep = si.on_wait[:-1], si.on_wait[-1:]
                    for w in extra:
                        nop = mybir.InstNoOp(
                            name=nc.get_next_instruction_name(),
                            ins=[],
                            outs=[],
                            engine=ins.engine,
                        )
                        nop.sync_info = mybir.SyncInfo(on_wait=[w], on_update=[])
                        new_insts.append(nop)
                        moved += 1
                    si.on_wait = keep
                new_insts.append(ins)
            bb.instructions = new_insts
    return moved


def _patched_drain_and_barrier(self, tick_clock, wait_clock):
    nop = self.nc.sync.nop(nofuse=True)
    wait_clock.add_sem_waits(nop.ins, ScopedClock({None: tick_clock.global_clock}))
    w = nop.ins.sync_info.on_wait if nop.ins.sync_info else []
    while w and len(w) > 1:
        cond = w.pop()
        n2 = self.nc.sync.nop(nofuse=True)
        if n2.ins.sync_info is None:
            n2.ins.sync_info = mybir.SyncInfo(on_wait=[], on_update=[])
        n2.ins.sync_info.on_wait.append(cond)
    self.nc.sync.drain()
    self.nc.all_engine_barrier()
    assert self.sems is not None
    popped = self.nc._tile_sem_poison_stack.pop()
    assert popped is self._sem_poison
    self.nc.clear_and_free_semaphores(list(self.sems.allocated().values()))
    self.nc.all_engine_barrier()


tile.TileContext._drain_and_barrier = _patched_drain_and_barrier

# SBUF cap: tile_utils caps at 192KB/partition; cayman has 208 usable.
try:
    import concourse.tile_utils as _tile_utils

    if getattr(_tile_utils, "max_sbuf_usage", None) is not None:
        _tile_utils.max_sbuf_usage = 204 * 1024
except Exception:
    pass


# ---------------------------------------------------------------------------
# Device program
# ---------------------------------------------------------------------------
class _Ctx:
    pass


def build_program(reps: int = 1, mode: str = "full", nsplit: int = NSPLIT):
    nc = bass.Bass()

    xT = nc.dram_tensor("xT", [B, C, T], FP16, kind="ExternalInput")
    wqT = nc.dram_tensor("wqT", [C, HPC * D], FP16, kind="ExternalInput")
    wkT = nc.dram_tensor("wkT", [C, HPC * D], FP16, kind="ExternalInput")
    wvT = nc.dram_tensor("wvT", [C, HPC * D], FP16, kind="ExternalInput")
    woT = nc.dram_tensor("woT", [C, C], FP16, kind="ExternalInput")
    cos_t = nc.dram_tensor("cos_t", [D // 2, T], FP16, kind="ExternalInput")
    sin_t = nc.dram_tensor("sin_t", [D // 2, T], FP16, kind="ExternalInput")
    maskc = nc.dram_tensor("maskc", [P, 896], FP16, kind="ExternalInput")
    ones_col = nc.dram_tensor("ones_col", [P, 1], FP16, kind="ExternalInput")
    ones_row = nc.dram_tensor("ones_row", [1, P], FP16, kind="ExternalInput")

    out_rows = nc.dram_tensor("out_rows", [NSLICE, C], FP, kind="ExternalOutput")

    c = _Ctx()
    c.nc, c.mode = nc, mode
    c.nsplit, c.seg = nsplit, NSLICE // nsplit
    c.xT, c.woT, c.out_rows = xT, woT, out_rows
    c.cos_t, c.sin_t = cos_t, sin_t

    with tile.TileContext(nc) as tc:
        c.tc = tc
        with (
            tc.tile_pool(name="const", bufs=1) as const,
            tc.tile_pool(name="wpool", bufs=1) as wpool,
            tc.tile_pool(name="xt", bufs=2) as xt_pool,
            tc.tile_pool(name="qkv", bufs=2) as qkv_pool,
            tc.tile_pool(name="ptile", bufs=5) as p_pool,
            tc.tile_pool(name="evict", bufs=2) as e_pool,
            tc.tile_pool(name="small", bufs=2) as s_pool,
            tc.tile_pool(name="dacc", bufs=2) as acc_pool,
            tc.tile_pool(name="ytp", bufs=2) as yt_pool,
            tc.tile_pool(name="wop", bufs=2) as wo_pool,
            tc.tile_pool(name="outp", bufs=2) as out_pool,
            tc.tile_pool(name="ps", bufs=1, space="PSUM") as ps,
        ):
            c.xt_pool, c.qkv_pool, c.p_pool = xt_pool, qkv_pool, p_pool
            c.e_pool, c.s_pool, c.acc_pool = e_pool, s_pool, acc_pool
            c.yt_pool, c.wo_pool, c.out_pool, c.ps = yt_pool, wo_pool, out_pool, ps

            c.mask_s = const.tile([P, 896], FP16)
            c.ones_c = const.tile([P, 1], FP16)
            c.ones_r = const.tile([1, P], FP16)
            nc.sync.dma_start(c.mask_s[:], maskc[:])
            nc.sync.dma_start(c.ones_c[:], ones_col[:])
            nc.sync.dma_start(c.ones_r[:], ones_row[:])

            c.wq_s, c.wk_s, c.wv_s = [], [], []
            for cc_i in range(NCC):
                tq = wpool.tile([P, HPC * D], FP16, tag=f"wq{cc_i}", name=f"wq{cc_i}")
                tk = wpool.tile([P, HPC * D], FP16, tag=f"wk{cc_i}", name=f"wk{cc_i}")
                tv = wpool.tile([P, HPC * D], FP16, tag=f"wv{cc_i}", name=f"wv{cc_i}")
                nc.sync.dma_start(tq[:], wqT[P * cc_i : P * (cc_i + 1), :])
                nc.sync.dma_start(tk[:], wkT[P * cc_i : P * (cc_i + 1), :])
                nc.sync.dma_start(tv[:], wvT[P * cc_i : P * (cc_i + 1), :])
                c.wq_s.append(tq)
                c.wk_s.append(tk)
                c.wv_s.append(tv)

            # per-rep A2A staging tensors
            c.ya_in, c.ya_out = {}, {}
            for rep in range(reps):
                c.ya_in[rep] = [
                    [nc.dram_tensor(f"ya_in_{rep}_{h}_{s}", [N_CORES * D, c.seg], FP16)
                     for s in range(nsplit)]
                    for h in range(HPC)
                ]
                c.ya_out[rep] = (
                    c.ya_in[rep] if mode == "nocc" else [
                        [nc.dram_tensor(f"ya_out_{rep}_{h}_{s}", [N_CORES * D, c.seg], FP16)
                         for s in range(nsplit)]
                        for h in range(HPC)
                    ]
                )

            phase3 = mode not in ("proj", "attn")
            for rep in range(reps):
                for b in range(B):
                    _emit_batch(c, rep, b)
                    # software-pipelined output projection of the previous rep
                    if b == 2 and rep > 0 and phase3:
                        _emit_outproj(c, rep - 1)
                if mode not in ("proj", "attn", "nocc"):
                    for s in range(nsplit):
                        for h in range(HPC):
                            nc.gpsimd.collective_compute(
                                "AllToAll",
                                mybir.AluOpType.bypass,
                                replica_groups=[list(range(N_CORES))],
                                ins=[c.ya_in[rep][h][s][:]],
                                outs=[c.ya_out[rep][h][s][:]],
                            )
            if phase3:
                _emit_outproj(c, reps - 1)

    moved = _fixup_multiwaits(nc)
    return nc, moved


def _emit_batch(c, rep, b):
    nc, mode = c.nc, c.mode
    HALF = D // 2

    # ---- QKV projections for batch b ----------------------------------
    qk_tiles = {}
    for pj in ("q", "k"):
        for h in range(HPC):
            qk_tiles[(pj, h)] = c.qkv_pool.tile(
                [P, T], FP16, tag=f"{pj}T{h}", name=f"{pj}T{h}_{rep}_{b}"
            )
    v_tiles = [
        c.qkv_pool.tile([P, HPC * D], FP16, tag=f"v{tch}", name=f"v{tch}_{rep}_{b}")
        for tch in range(T // P)
    ]

    for tg in range(T // TGP):
        cos_sl = c.s_pool.tile([HALF, TGP], FP16, tag="cosS", name=f"cos_{rep}_{b}_{tg}")
        sin_sl = c.s_pool.tile([HALF, TGP], FP16, tag="sinS", name=f"sin_{rep}_{b}_{tg}")
        nc.sync.dma_start(cos_sl[:], c.cos_t[:, TGP * tg : TGP * (tg + 1)])
        nc.sync.dma_start(sin_sl[:], c.sin_t[:, TGP * tg : TGP * (tg + 1)])
        xts = []
        for cc_i in range(NCC):
            xt = c.xt_pool.tile([P, TGP], FP16, tag=f"xt{cc_i}", name=f"xt{cc_i}_{rep}_{b}_{tg}")
            nc.sync.dma_start(
                xt[:], c.xT[b, P * cc_i : P * (cc_i + 1), TGP * tg : TGP * (tg + 1)]
            )
            xts.append(xt)

        # q, k: transposed orientation [d, t] with fused RoPE evict
        for pj, wt in (("q", c.wq_s), ("k", c.wk_s)):
            for h in range(HPC):
                pmm = c.ps.tile([P, 2 * TG], FP, tag="big2", bufs=2,
                                name=f"p{pj}{h}_{rep}_{b}_{tg}")
                for cc_i in range(NCC):
                    nc.tensor.matmul(
                        pmm[:, 0:TGP],
                        wt[cc_i][:, D * h : D * (h + 1)],
                        xts[cc_i][:],
                        start=(cc_i == 0),
                        stop=(cc_i == NCC - 1),
                    )
                dst = qk_tiles[(pj, h)]
                # evict PSUM->SBUF fp16 on Act (idle in this phase) so the
                # RoPE arithmetic runs in DVE's fast all-SBUF 2-byte mode;
                # two tiles so both DVE inputs sit at base partition 0
                evA = c.s_pool.tile([HALF, TGP], FP16, tag="ropeEA", name=f"evA_{rep}_{b}_{tg}")
                evB = c.s_pool.tile([HALF, TGP], FP16, tag="ropeEB", name=f"evB_{rep}_{b}_{tg}")
                nc.scalar.copy(evA[:], pmm[0:HALF, 0:TGP])
                nc.scalar.copy(evB[:], pmm[HALF:P, 0:TGP])
                t1 = c.s_pool.tile([HALF, TGP], FP16, tag="ropeA", bufs=1, name=f"t1_{rep}_{b}_{tg}")
                t2 = c.s_pool.tile([HALF, TGP], FP16, tag="ropeB", bufs=1, name=f"t2_{rep}_{b}_{tg}")
                x1 = evA[:]
                x2 = evB[:]
                dcol = dst[:, TGP * tg : TGP * (tg + 1)]
                nc.vector.tensor_mul(t1[:], x1, cos_sl[:])
                nc.vector.tensor_mul(t2[:], x2, sin_sl[:])
                nc.vector.tensor_sub(dcol[0:HALF, :], t1[:], t2[:])
                nc.vector.tensor_mul(t1[:], x1, sin_sl[:])
                nc.vector.tensor_mul(t2[:], x2, cos_sl[:])
                nc.vector.tensor_add(dcol[HALF:P, :], t1[:], t2[:])

        # v: natural orientation [t, d] for both heads
        for t4 in range(TGP // P):
            tch = (TGP * tg) // P + t4
            pv = c.ps.tile([P, HPC * D], FP, tag="misc", bufs=2, name=f"pv_{rep}_{b}_{tch}")
            for cc_i in range(NCC):
                nc.tensor.matmul(
                    pv[:],
                    xts[cc_i][:, P * t4 : P * (t4 + 1)],
                    c.wv_s[cc_i][:],
                    start=(cc_i == 0),
                    stop=(cc_i == NCC - 1),
                )
            nc.scalar.copy(v_tiles[tch][:], pv[:])

    if mode == "proj":
        sink = nc.dram_tensor(f"sink_{rep}_{b}", [P, 40 * 1024], FP16)
        for idx, ((pj, h), tl) in enumerate(qk_tiles.items()):
            nc.sync.dma_start(sink[:, idx * T : idx * T + T], tl[:])
        for tch, vt in enumerate(v_tiles):
            nc.sync.dma_start(
                sink[:, 17 * T + tch * HPC * D : 17 * T + (tch + 1) * HPC * D], vt[:]
            )
        return

    # ---- attention for batch b ----------------------------------------
    for h in range(HPC):
        qT = qk_tiles[("q", h)]
        kT = qk_tiles[("k", h)]
        for g in range(NTG):
            n_i = 4 * g + 4  # causal: tk chunks 0 .. 4g+3 (even count)
            po = c.ps.tile([P, TG], FP, tag="acc512", bufs=2, name=f"po_{rep}_{b}_{h}_{g}")
            acc = c.acc_pool.tile([P, TG], FP16, tag="dacc", name=f"acc_{rep}_{b}_{h}_{g}")
            for pi in range(n_i // 2):
                i0, i1 = 2 * pi, 2 * pi + 1
                # causally-live column range of each chunk
                r0 = max(0, P * i0 - TG * g)
                r1 = max(0, P * i1 - TG * g)
                pss = c.ps.tile([P, 2 * TG], FP, tag="big2", bufs=2,
                                name=f"ps_{rep}_{b}_{h}_{g}_{pi}")
                for half, (i, r) in enumerate(((i0, r0), (i1, r1))):
                    nc.tensor.matmul(
                        pss[:, TG * half + r : TG * (half + 1)],
                        kT[:, P * i : P * (i + 1)],
                        qT[:, TG * g + r : TG * (g + 1)],
                        start=True,
                        stop=True,
                    )
                pt = c.p_pool.tile([P, 2 * TG], FP16, tag="pT", name=f"pt_{rep}_{b}_{h}_{g}_{pi}")
                nc.scalar.activation(
                    pt[:, r0 : 2 * TG], pss[:, r0 : 2 * TG], EXP, scale=SCALE
                )
                # causal mask: only the first 128 live columns of a diagonal
                # chunk are partially masked; beyond that it is all-ones.
                for half, (i, r) in enumerate(((i0, r0), (i1, r1))):
                    if i >= 4 * g and "nomask" not in mode:
                        c0 = TG * half + r
                        w = min(P, TG - r)
                        nc.vector.tensor_mul(
                            pt[:, c0 : c0 + w], pt[:, c0 : c0 + w],
                            c.mask_s[:, 384 : 384 + w],
                        )
                # PV accumulation over live columns
                for half, (i, r) in enumerate(((i0, r0), (i1, r1))):
                    pth = pt[:, TG * half : TG * (half + 1)]
                    nc.tensor.matmul(
                        po[:, r:TG],
                        v_tiles[i][:, D * h : D * (h + 1)],
                        pth[:, r:TG],
                        start=(i == 0),
                        stop=(i == n_i - 1),
                    )
                # denominator partials on DVE (fp16; sums < 1e4)
                if pi == 0:
                    if r1 > 0:  # g==0: chunk1 live from col 128
                        nc.vector.tensor_copy(acc[:, 0:r1], pt[:, 0:r1])
                        nc.vector.tensor_add(
                            acc[:, r1:TG], pt[:, r1:TG], pt[:, TG + r1 : 2 * TG]
                        )
                    else:
                        nc.vector.tensor_add(acc[:], pt[:, 0:TG], pt[:, TG : 2 * TG])
                else:
                    nc.vector.tensor_add(acc[:, r0:TG], acc[:, r0:TG], pt[:, r0:TG])
                    nc.vector.tensor_add(
                        acc[:, r1:TG], acc[:, r1:TG], pt[:, TG + r1 : 2 * TG]
                    )
            # denominator: one partition-reduce matmul per group
            pd = c.ps.tile([1, TG], FP, tag="misc", bufs=2, name=f"pd_{rep}_{b}_{h}_{g}")
            nc.tensor.matmul(pd[:], c.ones_c[:], acc[:], start=True, stop=True)
            recip = c.s_pool.tile([1, TG], FP16, tag="recip", name=f"rc_{rep}_{b}_{h}_{g}")
            with nc.allow_low_precision(reason="softmax denom recip; values O(1e3)"):
                nc.vector.reciprocal(recip[:], pd[:])
            prb = c.ps.tile([P, TG], FP, tag="misc", bufs=2, name=f"prb_{rep}_{b}_{h}_{g}")
            nc.tensor.matmul(prb[:], c.ones_r[:], recip[:], start=True, stop=True)
            rb = c.e_pool.tile([P, TG], FP, tag="rb", name=f"rb_{rep}_{b}_{h}_{g}")
            nc.vector.tensor_copy(rb[:], prb[:])
            yt = c.e_pool.tile([P, TG], FP16, tag="yt", name=f"yt_{rep}_{b}_{h}_{g}")
            nc.vector.tensor_mul(yt[:], po[:], rb[:])
            # stage for A2A: shard j = n // NSLICE, segment s = (n % NSLICE) // SEG
            n0 = T * b + TG * g
            j = n0 // NSLICE
            col = n0 % NSLICE
            s = col // c.seg
            nc.sync.dma_start(
                c.ya_in[rep][h][s][D * j : D * (j + 1),
                                   col - s * c.seg : col - s * c.seg + TG], yt[:]
            )


def _emit_outproj(c, rep):
    nc = c.nc
    ya_out = c.ya_out[rep]
    seg_sz = c.seg
    cc_order = [HPC * j + h for h in range(HPC) for j in range(N_CORES)]
    for seg in range(c.nsplit):
        # wo for the first jg loads ahead of the collective-gated yts so the
        # DMA queue isn't head-blocked when the collective lands
        wos0 = {}
        for cc_i in range(NCC):
            wo = c.wo_pool.tile(
                [P, TG], FP16, tag=f"wo{cc_i}", name=f"wo{cc_i}_{rep}_{seg}_0"
            )
            nc.sync.dma_start(wo[:], c.woT[P * cc_i : P * (cc_i + 1), 0:TG])
            wos0[cc_i] = wo
        yts = {}
        for h in range(HPC):
            for j in range(N_CORES):
                cc_i = HPC * j + h
                yt = c.yt_pool.tile(
                    [P, seg_sz], FP16, tag=f"y{cc_i}", bufs=(2 if c.nsplit > 1 else 1),
                    name=f"y{cc_i}_{rep}_{seg}"
                )
                nc.sync.dma_start(yt[:], ya_out[h][seg][P * j : P * (j + 1), :])
                yts[cc_i] = yt
        for jg in range(C // TG):
            if jg == 0:
                wos = wos0
            else:
                wos = {}
                for cc_i in range(NCC):
                    wo = c.wo_pool.tile(
                        [P, TG], FP16, tag=f"wo{cc_i}", name=f"wo{cc_i}_{rep}_{seg}_{jg}"
                    )
                    nc.sync.dma_start(
                        wo[:], c.woT[P * cc_i : P * (cc_i + 1), TG * jg : TG * (jg + 1)]
                    )
                    wos[cc_i] = wo
            for nt in range(seg_sz // P):
                row = seg_sz * seg + P * nt
                pout = c.ps.tile(
                    [P, TG], FP, tag="acc512", bufs=2, name=f"pout_{rep}_{seg}_{jg}_{nt}"
                )
                for ci, cc_i in enumerate(cc_order):
                    nc.tensor.matmul(
                        pout[:],
                        yts[cc_i][:, P * nt : P * (nt + 1)],
                        wos[cc_i][:],
                        start=(ci == 0),
                        stop=(ci == NCC - 1),
                    )
                ot = c.out_pool.tile([P, TG], FP, tag="ot", name=f"ot_{rep}_{seg}_{jg}_{nt}")
                nc.scalar.copy(ot[:], pout[:])
                nc.sync.dma_start(
                    c.out_rows[row : row + P, TG * jg : TG * (jg + 1)], ot[:]
                )


# ---------------------------------------------------------------------------
# Host-side prep + execution
# ---------------------------------------------------------------------------
def _host_inputs(x, wq, wk, wv, wo):
    xT = np.ascontiguousarray(x.transpose(0, 2, 1)).astype(np.float16)
    woT = np.ascontiguousarray(wo.T).astype(np.float16)

    half = D // 2
    freqs = 1.0 / (10000.0 ** (np.arange(half, dtype=np.float32) / half))
    t = np.arange(T, dtype=np.float32)
    ang = freqs[:, None] * t[None, :]  # [half, T]
    cos_t = np.cos(ang).astype(np.float16)
    sin_t = np.sin(ang).astype(np.float16)

    # maskc[p, n] = 1.0 iff p <= n - 384  (sliced per diagonal offset)
    pp = np.arange(P)[:, None]
    nn = np.arange(896)[None, :]
    maskc = (pp <= nn - 384).astype(np.float16)

    ones_col = np.ones((P, 1), dtype=np.float16)
    ones_row = np.ones((1, P), dtype=np.float16)

    common = dict(
        xT=xT, woT=woT, cos_t=cos_t, sin_t=sin_t, maskc=maskc,
        ones_col=ones_col, ones_row=ones_row,
    )
    in_maps = []
    for r in range(N_CORES):
        rows = slice(HPC * D * r, HPC * D * (r + 1))
        in_maps.append(
            dict(
                common,
                wqT=np.ascontiguousarray(wq[rows, :].T).astype(np.float16),
                wkT=np.ascontiguousarray(wk[rows, :].T).astype(np.float16),
                wvT=np.ascontiguousarray(wv[rows, :].T).astype(np.float16),
            )
        )
    return in_maps


_CACHED = {}


def _get_program(reps=1):
    if reps not in _CACHED:
        _CACHED[reps] = build_program(reps)[0]
    return _CACHED[reps]


def kernel(x, wq, wk, wv, wo):
    nc = _get_program(1)
    in_maps = _host_inputs(
        np.asarray(x, dtype=np.float32),
        np.asarray(wq, dtype=np.float32),
        np.asarray(wk, dtype=np.float32),
        np.asarray(wv, dtype=np.float32),
        np.asarray(wo, dtype=np.float32),
    )
    res = run_bass_kernel_spmd(nc, in_maps, list(range(N_CORES)))
    out = np.concatenate([res.results[r]["out_rows"] for r in range(N_CORES)], axis=0)
    return out.reshape(B, T, C)
